# revision 33
# baseline (speedup 1.0000x reference)
"""Embedding lookup (gather) kernel for Trainium2, 8 NeuronCores.

Problem: out[i] = table[value_tensors[i]] for 212992 indices into a
[1M, 128] f32 table, reshaped to [8192, 26, 128]. (row_offsets is
arange, so the CSR segment-sum is the identity; a host-side fallback
handles the general case.)

Sharding: model-parallel by table row (range partition). The table is
split into 32 range bins of 31250 rows; core c owns bins 4c..4c+3.
The host routes each lookup index to its owning bin, each core gathers
its rows on-device with the SWDGE dma_gather instruction, and the host
scatters the gathered rows back to the original positions (the
"all-to-all" step of HugeCTR's localized embedding, done at unshard
time).

Perf strategy. Measured path: f32 one-desc-per-row baseline 126us;
fp16 tables/outputs halve both HBM directions -> 88us; the gather is
then DESCRIPTOR-rate bound (~100 descs/us/queue at 256B or 512B;
engine-contiguous idx permutation measured neutral -> fixed per-desc
cost, not HBM page locality). So:
  - fp16 table + fp16 output, host upconverts (max rel err 2^-11 vs
    the 2e-2 gate); dedupe indices on host (~10% repeats).
  - WINDOW DESCRIPTORS: the sorted unique rows are greedily covered by
    windows of <= 4 consecutive table rows; each window is ONE
    descriptor (elem_size = span*128 over an overlapping-stride AP
    view with elem_step=128). ~37% fewer descriptors than
    one-per-row at ~30ns/desc fixed cost.
  - runtime num_idxs via batched reg_load from a per-core count table
    + negative idx padding: pad slots generate no descriptors, so
    per-bin/class padding (compile-time shapes shared across cores) is
    nearly free. Every chunk keeps >= 16 non-negative idxs so all 16
    engine rings still fire their completion-semaphore descriptor.
  - warmup dma_gather per queue overlapped with the idx load (first
    SWDGE gather on a cold queue measured ~9us of init).
  - per-bin idx loads; few semaphores; whole-bin writes alternating
    between the two HWDGE rings (Sync/Scalar) overlap the gathers.

dma_gather layout (probed on HW, incl. runtime-reg + negative-pad +
overlapping-window-view semantics via probe_reg.py / probe_win.py):
indices are int16, wrapped over 16 partitions (ordinal i reads
idx[i % 16, i // 16]) and replicated to all 8 Q7-core partition
groups; gathered ordinal i lands at dst[i % 128, i // 128]; negative
idxs at the end generate no descriptors.
"""

import time

import numpy as np

VOCAB = 1_000_000
BATCH = 8192
SLOTS = 26
VEC = 128
NCORES = 8
NSUB = 4  # sub-shards (bins) per core; int16 gather idx needs rows <= 32767
RSUB = VOCAB // (NCORES * NSUB)  # 31250 rows per bin
SHARD = RSUB * NSUB  # 125000 rows per core
P = 128
# Idxs per dma_gather: 896 -> 56 data descs + 1 sem desc per engine ring,
# safely under the 64-descriptor packet ceiling. One fused call per
# (bin, class) with single_packet=False measured 27% SLOWER: the Pool
# stream stalls inside the big call's descriptor emission and the other
# three queues starve — chunked calls keep all 4 queues' DGE interleaved.
# (CH=384 for 2-packet ring depth fails reproducibly at compile/load
# even with the reg_load batch split to <=24 wide — the larger program
# itself breaks the toolchain. Do not retry without a toolchain fix.)
CH = 896
MXSPAN = 4
NCLS = MXSPAN  # class c gathers windows of span c+1 rows
ELEM = [(c + 1) * VEC for c in range(NCLS)]

LAST_RUN = None  # BassKernelResults of the most recent device run (for test.py)


def _chunks_of(N: int):
    out = []
    o = 0
    while o < N:
        out.append((o, min(CH, N - o)))
        o += CH
    return out


def _windows(rows: np.ndarray):
    """Greedy cover of sorted unique local rows by windows of <= MXSPAN
    consecutive table rows (optimal interval count).

    Returns per span class c (span = c+1):
      starts[c]: window start rows
      pos[c], w[c], off[c]: for each covered unique -> its bin-relative
      position, window ordinal within the class, and row offset.
    """
    n = len(rows)
    starts = [np.empty(0, np.int64) for _ in range(NCLS)]
    pos = [np.empty(0, np.int64) for _ in range(NCLS)]
    wloc = [np.empty(0, np.int64) for _ in range(NCLS)]
    off = [np.empty(0, np.int64) for _ in range(NCLS)]
    if n == 0:
        return starts, pos, wloc, off
    nxt = np.searchsorted(rows, rows + MXSPAN)
    si = []
    i = 0
    while i < n:
        si.append(i)
        i = nxt[i]
    si = np.asarray(si)
    ei = np.append(si[1:], n)
    span = rows[ei - 1] - rows[si] + 1  # 1..MXSPAN
    nwin = len(si)
    wid = np.repeat(np.arange(nwin), ei - si)
    offs = rows - rows[si][wid]
    allpos = np.arange(n)
    for c in range(NCLS):
        selw = span == c + 1
        starts[c] = rows[si[selw]]
        wl = np.cumsum(selw) - 1  # class-local ordinal per window
        selu = selw[wid]
        pos[c] = allpos[selu]
        wloc[c] = wl[wid[selu]]
        off[c] = offs[selu]
    return starts, pos, wloc, off


def _build_program(NCL: list, ncalls_bin: int):
    """One SPMD program for all 8 cores. NCL[c] = padded idx slots for
    class c per bin (multiples of 128, identical across cores/bins).

    Per core:
      shard [SHARD, VEC] fp16   - this core's 4 bins, concatenated
      idx   [P, ICOLS] i16      - [8 warm cols][bin0 c0..c3][bin1 ...]
      cnt   [1, NCALL] i32      - per-gather-call runtime num_idxs
      out   [P, NSUB*W] fp16    - W = per-bin output cols
    """
    import bass_rust
    import concourse.bacc as bacc
    from concourse import mybir
    from concourse.library_config import mlp

    chunks = [_chunks_of(NCL[c]) for c in range(NCLS)]
    icols_bin = sum(NCL) // 16
    ccols = [(NCL[c] // 128) * ELEM[c] for c in range(NCLS)]
    roff = [0] + list(np.cumsum(ccols))[:-1]
    W = sum(ccols)
    ICOLS = 8 + NSUB * icols_bin
    NCALL = NSUB * ncalls_bin

    nc = bacc.Bacc("TRN2", num_swdge_queues=4)
    shard = nc.declare_dram_parameter(
        "shard", [SHARD, VEC], mybir.dt.float16, isOutput=False
    )
    idx = nc.declare_dram_parameter("idx", [P, ICOLS], mybir.dt.int16, isOutput=False)
    cnt = nc.declare_dram_parameter("cnt", [1, NCALL], mybir.dt.int32, isOutput=False)
    out = nc.declare_dram_parameter(
        "out", [P, NSUB * W], mybir.dt.float16, isOutput=True
    )

    sem_in = nc.alloc_semaphore("sem_in")
    sem_warm = nc.alloc_semaphore("sem_warm")
    # per-bin gather sems; bin 3's multi-row classes get their own so
    # the post-gather write tail is only that region. (Finer write
    # splits and other sem arrangements measured 1-6us SLOWER.)
    sem_g = [nc.alloc_semaphore(f"sem_g{s}") for s in range(NSUB + 1)]
    sem_out = nc.alloc_semaphore()

    idx_sb = nc.alloc_sbuf_tensor("idx_sb", [P, ICOLS], mybir.dt.int16).ap()
    cnt_sb = nc.alloc_sbuf_tensor("cnt_sb", [1, NCALL], mybir.dt.int32).ap()
    warm_out = nc.alloc_sbuf_tensor("warm_out", [P, 1, VEC], mybir.dt.float16).ap()
    g_bufs = [
        nc.alloc_sbuf_tensor(f"g{s}", [P, W], mybir.dt.float16).ap()
        for s in range(NSUB)
    ]

    nc.gpsimd.load_library(mlp)
    # cnt + warm idx cols first (threshold 32), then one idx DMA per bin
    # on the same HWDGE ring: FIFO completion => sem_in thresholds.
    nc.sync.dma_start(out=cnt_sb[:], in_=cnt[:, :]).then_inc(sem_in, 16)
    nc.sync.dma_start(out=idx_sb[:, 0:8], in_=idx[:, 0:8]).then_inc(sem_in, 16)
    for s in range(NSUB):
        a, b = 8 + s * icols_bin, 8 + (s + 1) * icols_bin
        nc.sync.dma_start(out=idx_sb[:, a:b], in_=idx[:, a:b]).then_inc(sem_in, 16)

    warm_reg = nc.gpsimd.to_reg(128)
    cregs = [nc.gpsimd.alloc_register(name=f"creg{t}") for t in range(NCALL)]
    nc.gpsimd.wait_ge(sem_in, 16)
    # Batched loads, <= 24 regs each (52-wide measured failing to lower).
    for i in range(0, NCALL, 24):
        j = min(i + 24, NCALL)
        nc.gpsimd.reg_load(cregs[i:j], cnt_sb[0:1, i:j])

    # Warmup: one tiny gather (row 0 x128) while the idx DMAs are still
    # in flight; absorbs the ~9us cold SWDGE init (global, not per-queue:
    # in the un-warmed baseline only the first gather overall paid it).
    nc.gpsimd.wait_ge(sem_in, 32)
    nc.gpsimd.dma_gather(
        warm_out[:, :, :],
        shard[0:RSUB, :],
        idx_sb[:, 0:8],
        128,
        warm_reg,
        VEC,
        queue_num=0,
    ).then_inc(sem_warm, 16)

    qn = 0
    t = 0
    for s in range(NSUB):
        nc.gpsimd.wait_ge(sem_in, 32 + 16 * (s + 1))
        for c in range(NCLS):
            L = c + 1
            view = shard[s * RSUB : s * RSUB + (RSUB - L + 1), :].copy()
            view.ap = bass_rust.VecI64Pair([[VEC, RSUB - L + 1], [1, ELEM[c]]])
            ibase = 8 + s * icols_bin + sum(NCL[:c]) // 16
            for o, sz in chunks[c]:
                sem = sem_g[s] if (s < NSUB - 1 or c == 0) else sem_g[NSUB]
                dst = g_bufs[s][
                    :,
                    roff[c] + (o // 128) * ELEM[c] : roff[c]
                    + ((o + sz) // 128) * ELEM[c],
                ].rearrange("p (k e) -> p k e", e=ELEM[c])
                nc.gpsimd.dma_gather(
                    dst,
                    view,
                    idx_sb[:, ibase + o // 16 : ibase + (o + sz) // 16],
                    sz,
                    cregs[t],
                    ELEM[c],
                    elem_step=VEC,
                    queue_num=qn % 4,
                ).then_inc(sem, 16)
                qn += 1
                t += 1
    assert t == NCALL

    nch0 = len(chunks[0])
    nrest = ncalls_bin - nch0
    # Whole-bin writes (fat ~18KB/partition descriptors; fine-grained
    # early writes measured slower — extra packets disturb the gather
    # drain), alternating between the two HWDGE rings; bin 3 split
    # [class 0 | classes 1..3] so the tail waits only on its region.
    writes = []  # (engine_idx, bin, sem, need, col0, col1)
    for s in range(NSUB - 1):
        writes.append((s % 2, s, sem_g[s], 16 * ncalls_bin, 0, W))
    # The final (bin3 multi-row) write rides Scalar: its ring is empty by
    # the time the last gathers land, while Sync's FIFO still drains bins
    # 0/2 — on Sync the tail write queued ~3us behind that backlog.
    writes.append((0, NSUB - 1, sem_g[NSUB - 1], 16 * nch0, 0, ccols[0]))
    writes.append((1, NSUB - 1, sem_g[NSUB], 16 * nrest, ccols[0], W))
    for ei, s, sem, need, c0, c1 in writes:
        eng = nc.sync if ei == 0 else nc.scalar
        eng.wait_ge(sem, need)
        eng.dma_start(
            out=out[:, s * W + c0 : s * W + c1], in_=g_bufs[s][:, c0:c1]
        ).then_inc(sem_out, 16)
    nc.sync.wait_ge(sem_out, 16 * len(writes))
    nc.sync.wait_ge(sem_warm, 16)
    nc.finalize()
    return nc


def _wrap_cols(vals: np.ndarray, N: int, ecount: int) -> np.ndarray:
    """int16 idx block [16, N//16]: element i at [i%16, i//16]; slots
    [len(vals), ecount) hold 0 (valid row, gathered then ignored), slots
    [ecount, N) hold -1 (skipped by the ucode)."""
    li = np.full(N, -1, np.int16)
    li[:ecount] = 0
    li[: len(vals)] = vals.astype(np.int16)
    return li.reshape(N // 16, 16).T


def _gather_on_device(table_f16: np.ndarray, uniq: np.ndarray) -> np.ndarray:
    """emb[i] = table[uniq[i]] (fp16) computed on 8 NeuronCores."""
    global LAST_RUN
    from concourse.bass_utils import run_bass_kernel_spmd

    total = uniq.shape[0]
    nbins = NCORES * NSUB
    bin_id = (uniq // RSUB).astype(np.int32)
    local = (uniq - bin_id.astype(np.int64) * RSUB).astype(np.int32)
    counts = np.bincount(bin_id, minlength=nbins)
    assert counts.sum() == total
    bin_start = np.concatenate(([0], np.cumsum(counts)))

    dec = []  # dec[b] = (starts, pos, w, off) per class
    ncls_max = [0] * NCLS
    for b in range(nbins):
        parts = _windows(local[bin_start[b] : bin_start[b + 1]])
        dec.append(parts)
        for c in range(NCLS):
            ncls_max[c] = max(ncls_max[c], len(parts[0][c]))
    NCL = [max(P, ((m + P - 1) // P) * P) for m in ncls_max]
    chunks = [_chunks_of(NCL[c]) for c in range(NCLS)]
    ncalls_bin = sum(len(ch) for ch in chunks)
    icols_bin = sum(NCL) // 16
    ccols = [(NCL[c] // 128) * ELEM[c] for c in range(NCLS)]
    roff = [0] + list(np.cumsum(ccols))[:-1]
    W = sum(ccols)

    in_maps = []
    for core in range(NCORES):
        blocks = [np.zeros((16, 8), np.int16)]  # warm cols
        cvals = []
        for s in range(NSUB):
            b = core * NSUB + s
            starts = dec[b][0]
            for c in range(NCLS):
                n = len(starts[c])
                o_last = chunks[c][-1][0]
                ecount = max(n, o_last + 16)
                blocks.append(_wrap_cols(starts[c], NCL[c], ecount))
                for o, sz in chunks[c]:
                    cvals.append(min(ecount - o, sz))
        in_maps.append(
            {
                "shard": np.ascontiguousarray(
                    table_f16[core * SHARD : (core + 1) * SHARD]
                ),
                "idx": np.ascontiguousarray(
                    np.tile(np.concatenate(blocks, axis=1), (8, 1))
                ),
                "cnt": np.array([cvals], np.int32),
            }
        )

    # The shared device occasionally wedges transiently
    # (NRT_EXEC_UNIT_UNRECOVERABLE / profile-stop rc=-1); a fresh attempt
    # after a short pause recovers it.
    for attempt in range(3):
        try:
            nc = _build_program(NCL, ncalls_bin)
            LAST_RUN = run_bass_kernel_spmd(nc, in_maps, list(range(NCORES)))
            break
        except Exception:
            if attempt == 2:
                raise
            time.sleep(10)
    res = LAST_RUN.results

    emb = np.empty((total, VEC), np.float16)
    for core in range(NCORES):
        o = np.asarray(res[core]["out"])
        for s in range(NSUB):
            b = core * NSUB + s
            bs = bin_start[b]
            _, pos, wloc, off = dec[b]
            reg = o[:, s * W : (s + 1) * W]
            for c in range(NCLS):
                if len(pos[c]) == 0:
                    continue
                nw = int(wloc[c].max()) + 1
                seg = reg[:, roff[c] : roff[c] + ccols[c]]
                wins = (
                    seg.reshape(P, NCL[c] // 128, ELEM[c])
                    .transpose(1, 0, 2)
                    .reshape(-1, ELEM[c])[:nw]
                    .reshape(nw, c + 1, VEC)
                )
                emb[bs + pos[c]] = wins[wloc[c], off[c]]
    return emb


def kernel(table, row_offsets, value_tensors, nnz_array=None, output_shape=None):
    table = np.asarray(table, dtype=np.float32)
    assert table.shape == (VOCAB, VEC)
    v = np.asarray(value_tensors).astype(np.int64).ravel()
    total = v.shape[0]

    table_f16 = table.astype(np.float16)
    uniq, inverse = np.unique(v, return_inverse=True)
    emb_u = _gather_on_device(table_f16, uniq)
    emb = emb_u[inverse].astype(np.float32)

    n_rows = BATCH * SLOTS
    ro = np.asarray(row_offsets).astype(np.int64).ravel()
    if total == n_rows and np.array_equal(ro, np.arange(total + 1)):
        return emb.reshape(BATCH, SLOTS, VEC)
    # General CSR fallback (never hit with the reference's arange offsets):
    # sum-combine values per segment on the host.
    seg = np.searchsorted(ro, np.arange(total), side="right") - 1
    combined = np.zeros((n_rows, VEC), np.float32)
    np.add.at(combined, seg, emb)
    return combined.reshape(BATCH, SLOTS, VEC)
